# revision 1
# baseline (speedup 1.0000x reference)
"""Trainium2 Bass kernel: causal multi-head attention (B=4,S=2048,D=1024,H=16).

Sharding (8 cores, no collectives): core c -> batch b=c//2, q-half h=c%2.
Each core computes all 16 heads for 8 interleaved query tiles of 128 rows
(abs q-tile t = 2*j + h for local slot j), plus full K/V for its batch,
and the full fc_out for its own query rows.  The host scatters the 8
per-core [1024,1024] outputs back into [4,2048,1024].

Device pipeline per core (all matmuls bf16, f32 accumulation):
  P1: Q/K/V projections (stationary x^T blocks, moving per-head weights),
      PSUM->SBUF cast + bias, DMA-xbar transposes to build Q^T/K^T.
  P2: per (head, k-tile): scores^T = K^T.T @ Q^T -> PSUM, exp via ScalarE
      (scale=1/8 folded in), 0/1 mask multiply on "mixed" tiles only,
      out^T accumulation with ones-augmented V (row 64 = softmax denom).
      Normalization by the reciprocal of the denominator at head end.
  P3: fc_out = concat^T.T @ Wo + bo for the local query rows.

The program is specialized at build time to the mask's block structure
(skip all-zero blocks / skip masking on all-ones blocks); this is computed
from the actual mask input, so it stays correct for any mask.
"""

import os
import numpy as np
import ml_dtypes

import concourse.bass as bass
import concourse.mybir as mybir
import concourse.tile as tile
from concourse import bacc
from concourse.bass_utils import run_bass_kernel_spmd

B, S, D, H, HD = 4, 2048, 1024, 16, 64
N_CORES = 8
ST = 128               # tile edge (partition size)
NKT = S // ST          # 16 key tiles
NJ = 8                 # local query slots per core (8*128 = 1024 rows)
NDC = D // ST          # 8 contraction chunks
NG = H // 2            # 8 head pairs (2 heads packed per 128 partitions)

F32 = mybir.dt.float32
BF16 = mybir.dt.bfloat16


def _classify(mask: np.ndarray):
    """Block structure of the mask, unioned over the two q-halves.

    Returns (cls[NJ][NKT] in {0 skip,1 full,2 mixed}, mixed list [(j,k)]).
    """
    cls = np.zeros((NJ, NKT), dtype=int)
    for j in range(NJ):
        for k in range(NKT):
            blocks = [
                mask[(2 * j + h) * ST:(2 * j + h + 1) * ST, k * ST:(k + 1) * ST]
                for h in (0, 1)
            ]
            if all((b != 0).all() for b in blocks):
                cls[j, k] = 1
            elif all((b == 0).all() for b in blocks):
                cls[j, k] = 0
            else:
                cls[j, k] = 2
        # close interior holes so every slot's computed k-range is contiguous
        nz = np.nonzero(cls[j])[0]
        if len(nz):
            for k in range(nz[0], nz[-1] + 1):
                if cls[j, k] == 0:
                    cls[j, k] = 2
    mixed = [(j, k) for j in range(NJ) for k in range(NKT) if cls[j, k] == 2]
    return cls, mixed


def _build(cls, mixed, n_maskt):
    """Build the (uniform, SPMD) Bass program for one core's shard."""
    nc = bacc.Bacc("TRN2", target_bir_lowering=False, debug=False,
                   num_devices=N_CORES)

    x_d = nc.dram_tensor("x", [S, D], F32, kind="ExternalInput")
    xq_d = nc.dram_tensor("xq", [NJ * ST, D], F32, kind="ExternalInput")
    wq_d = nc.dram_tensor("wq", [H, D, HD], F32, kind="ExternalInput")
    wk_d = nc.dram_tensor("wk", [H, D, HD], F32, kind="ExternalInput")
    wv_d = nc.dram_tensor("wv", [H, D, HD], F32, kind="ExternalInput")
    wo_d = nc.dram_tensor("wo", [D, D], F32, kind="ExternalInput")
    bq_d = nc.dram_tensor("bq", [H, HD], F32, kind="ExternalInput")
    bk_d = nc.dram_tensor("bk", [H, HD], F32, kind="ExternalInput")
    bv_d = nc.dram_tensor("bv", [H, HD], F32, kind="ExternalInput")
    bo_d = nc.dram_tensor("bo", [D], F32, kind="ExternalInput")
    mt_d = nc.dram_tensor("maskt", [n_maskt, ST, ST], BF16, kind="ExternalInput")
    out_d = nc.dram_tensor("out", [NJ * ST, D], F32, kind="ExternalOutput")

    mixed_idx = {jk: i for i, jk in enumerate(mixed)}
    # per-k slot spans and per-slot k ranges
    slots_k = [[j for j in range(NJ) if cls[j, k]] for k in range(NKT)]
    kfirst = {}
    klast = {}
    for j in range(NJ):
        ks = [k for k in range(NKT) if cls[j, k]]
        if ks:
            kfirst[j], klast[j] = ks[0], ks[-1]

    NB = NJ // 4  # PSUM 512-col banks per po tile (2)
    NSG = NKT // 4  # 4 s-groups of 512 rows
    bank_slots = [[j for j in range(4 * b_, 4 * b_ + 4) if j in kfirst]
                  for b_ in range(NB)]
    bklast = {b_: max(klast[j] for j in bank_slots[b_])
              for b_ in range(NB) if bank_slots[b_]}
    bank_fast = {b_: len({kfirst[j] for j in bank_slots[b_]}) == 1
                 for b_ in range(NB) if bank_slots[b_]}

    from concourse.masks import make_identity

    with tile.TileContext(nc) as tc:
        with (
            tc.tile_pool(name="persist", bufs=1) as pp,      # lives whole kernel
        ):
            # ---- persistent SBUF tensors -------------------------------
            kt_t = [pp.tile([ST, S], BF16, name=f"ktg{g}", tag=f"ktg{g}")
                    for g in range(NG)]
            qt_t = [pp.tile([ST, NJ * ST], BF16, name=f"qtg{g}", tag=f"qtg{g}")
                    for g in range(NG)]
            vb = pp.tile([ST, NKT, H, HD + 1], BF16, name="vb", tag="vb")
            bob = pp.tile([ST, D], F32, name="bob", tag="bob")
            ident = pp.tile([ST, ST], BF16, name="ident", tag="ident")

            nc.vector.memset(vb[:, :, :, HD:HD + 1], 1.0)
            make_identity(nc, ident[:, :])
            bo_ap = bo_d.ap()
            nc.sync.dma_start(
                bob[:, :],
                bass.AP(tensor=bo_ap.tensor, offset=bo_ap.offset,
                        ap=[[0, ST]] + list(bo_ap.ap)))

            def load_bias_pair(pool, bias_d, name):
                # [128, NG] f32: partition = (h%2)*64+e, column = pair idx
                t = pool.tile([ST, NG], F32, name=name, tag=name, bufs=1)
                src = bias_d.ap()
                nc.scalar.dma_start(
                    t[:, :],
                    bass.AP(tensor=src.tensor, offset=src.offset,
                            ap=[[1, ST], [ST, NG]]))
                return t

            def load_w_pair(pool, w_d, tag):
                # [128, NDC, NG, 128]: stationary block for K^T/Q^T projection
                t = pool.tile([ST, NDC, NG, ST], BF16, name=tag, tag=tag, bufs=1)
                for h in range(H):
                    src = w_d.ap()[h].rearrange("(c p) e -> p c e", p=ST)
                    wstg = pool.tile([ST, NDC, HD], F32, tag="wstg")
                    nc.scalar.dma_start(wstg[:, :, :], src)
                    nc.vector.tensor_copy(
                        t[:, :, h // 2, (h % 2) * HD:(h % 2) * HD + HD],
                        wstg[:, :, :])
                return t

            def load_w_flat(pool, w_d, tag):
                # [128, NDC, H, HD]: moving operand for V projection
                t = pool.tile([ST, NDC, H, HD], BF16, name=tag, tag=tag, bufs=1)
                for h in range(H):
                    src = w_d.ap()[h].rearrange("(c p) e -> p c e", p=ST)
                    wstg = pool.tile([ST, NDC, HD], F32, tag="wstg")
                    nc.scalar.dma_start(wstg[:, :, :], src)
                    nc.vector.tensor_copy(t[:, :, h, :], wstg[:, :, :])
                return t

            # ---- phase 1: x^T, V, K^T, Q^T -----------------------------
            with (
                tc.tile_pool(name="p1a", bufs=2) as p1a,
                tc.tile_pool(name="pw", bufs=1) as pw,
            ):
                wkp = load_w_pair(pw, wk_d, "wkp")
                bkp = load_bias_pair(pw, bk_d, "bkp")
                bqp = load_bias_pair(pw, bq_d, "bqp")
                bvf = pw.tile([ST, H, HD], F32, name="bvf", tag="bvf", bufs=1)
                srcv = bv_d.ap()
                nc.sync.dma_start(
                    bvf[:, :, :],
                    bass.AP(tensor=srcv.tensor, offset=srcv.offset,
                            ap=[[0, ST]] + list(srcv.ap)))
                xt = {}
                with (
                    tc.tile_pool(name="xtp", bufs=1, side="right") as xtp,
                ):
                  for c in range(NDC):
                    for sg in range(NSG):
                        xt[c, sg] = xtp.tile([ST, 512], BF16,
                                             name=f"xt{c}_{sg}", tag=f"xt{c}_{sg}")
                  with (
                    tc.tile_pool(name="pv", bufs=1) as pv,
                    tc.tile_pool(name="ppsv", bufs=2, space="PSUM") as ppsa,
                    tc.tile_pool(name="ppst", bufs=3, space="PSUM") as ppst,
                  ):
                    wvb = load_w_flat(pv, wv_d, "wvb")
                    for sg in range(NSG):
                        for st in range(4 * sg, 4 * sg + 4):
                            so = (st % 4) * ST
                            xf = p1a.tile([ST, D], F32, tag="xf")
                            nc.sync.dma_start(
                                xf[:, :], x_d.ap()[st * ST:(st + 1) * ST, :])
                            xb = p1a.tile([ST, D], BF16, tag="xb")
                            nc.vector.tensor_copy(xb[:, :], xf[:, :])
                            for c in range(NDC):
                                pst = ppst.tile([ST, ST], BF16, tag="pst")
                                nc.tensor.transpose(
                                    pst[:, :], xb[:, c * ST:(c + 1) * ST],
                                    ident[:, :])
                                nc.scalar.copy(xt[c, sg][:, so:so + ST],
                                               pst[:, :])
                        for st in range(4 * sg, 4 * sg + 4):
                            so = (st % 4) * ST
                            psv = ppsa.tile([ST, H * HD], F32, tag="psv")
                            for c in range(NDC):
                                for n in range(2):
                                    nc.tensor.matmul(
                                        psv[:, n * 512:(n + 1) * 512],
                                        xt[c, sg][:, so:so + ST],
                                        wvb[:, c, 8 * n:8 * n + 8, :],
                                        start=(c == 0), stop=(c == NDC - 1))
                            nc.vector.tensor_add(
                                vb[:, st, :, 0:HD],
                                psv[:, :].rearrange("p (h e) -> p h e", h=H),
                                bvf[:, :, :])

                # Q^T: from the host-fed local query rows (xq), via the
                # same PE-transpose path, weight-pair stationary.
                NQG = NJ // 4
                with (
                    tc.tile_pool(name="pq", bufs=1) as pq,
                    tc.tile_pool(name="xqtp", bufs=1) as xqtp,
                    tc.tile_pool(name="ppsq", bufs=2, space="PSUM") as ppsq,
                    tc.tile_pool(name="ppstq", bufs=3, space="PSUM") as ppstq,
                ):
                    wqp = load_w_pair(pq, wq_d, "wqp")
                    xqt = {}
                    for c in range(NDC):
                        for sg in range(NQG):
                            xqt[c, sg] = xqtp.tile([ST, 512], BF16,
                                                   name=f"xqt{c}_{sg}",
                                                   tag=f"xqt{c}_{sg}")
                    for jl in range(NJ):
                        sg, so = jl // 4, (jl % 4) * ST
                        xf = p1a.tile([ST, D], F32, tag="xf")
                        nc.sync.dma_start(xf[:, :],
                                          xq_d.ap()[jl * ST:(jl + 1) * ST, :])
                        xb = p1a.tile([ST, D], BF16, tag="xb")
                        nc.vector.tensor_copy(xb[:, :], xf[:, :])
                        for c in range(NDC):
                            pst = ppstq.tile([ST, ST], BF16, tag="pstq")
                            nc.tensor.transpose(
                                pst[:, :], xb[:, c * ST:(c + 1) * ST], ident[:, :])
                            nc.vector.tensor_copy(xqt[c, sg][:, so:so + ST],
                                                  pst[:, :])
                    for g in range(NG):
                        psq = [ppsq.tile([ST, 512], F32, name=f"psq{sg}",
                                         tag=f"psq{sg}") for sg in range(NQG)]
                        for c in range(NDC):
                            for sg in range(NQG):
                                nc.tensor.matmul(
                                    psq[sg][:, :],
                                    wqp[:, c, g, :],
                                    xqt[c, sg][:, :],
                                    start=(c == 0), stop=(c == NDC - 1))
                        for sg in range(NQG):
                            nc.vector.tensor_scalar(
                                qt_t[g][:, sg * 512:(sg + 1) * 512],
                                psq[sg][:, :], bqp[:, g:g + 1], None,
                                mybir.AluOpType.add)

                # K^T: weight-pair stationary, x^T moving
                with (
                    tc.tile_pool(name="ppsk", bufs=2, space="PSUM") as ppsk,
                ):
                    for g in range(NG):
                        psk = [ppsk.tile([ST, 512], F32, name=f"psk{sg}",
                                         tag=f"psk{sg}") for sg in range(NSG)]
                        for c in range(NDC):
                            for sg in range(NSG):
                                nc.tensor.matmul(
                                    psk[sg][:, :],
                                    wkp[:, c, g, :],
                                    xt[c, sg][:, :],
                                    start=(c == 0), stop=(c == NDC - 1))
                        for sg in range(NSG):
                            nc.vector.tensor_scalar(
                                kt_t[g][:, sg * 512:(sg + 1) * 512],
                                psk[sg][:, :], bkp[:, g:g + 1], None,
                                mybir.AluOpType.add)

            # ---- phase 2: attention ------------------------------------
            late_cm = tc.tile_pool(name="late", bufs=1)
            late = late_cm.__enter__()
            cat = [late.tile([ST, NJ * ST], BF16, name=f"catg{g}",
                             tag=f"catg{g}") for g in range(NG)]
            mtb = late.tile([ST, max(n_maskt, 1), ST], BF16, name="mtb",
                            tag="mtb")
            wob = late.tile([ST, NDC, D], BF16, name="wob", tag="wob")
            nc.sync.dma_start(mtb[:, :, :], mt_d.ap().rearrange("m p f -> p m f"))
            with (
                tc.tile_pool(name="p2s", bufs=4) as p2s,
                tc.tile_pool(name="ldp", bufs=1, space="DRAM") as ldp,
                tc.tile_pool(name="pss", bufs=4, space="PSUM") as pss,
                tc.tile_pool(name="pso", bufs=2, space="PSUM") as pso,
            ):
                for c in range(NDC):
                    wstg = p2s.tile([ST, D], F32, tag="wstg3", bufs=2)
                    nc.sync.dma_start(wstg[:, :],
                                      wo_d.ap()[c * ST:(c + 1) * ST, :])
                    nc.vector.tensor_copy(wob[:, c, :], wstg[:, :])
                ldram = ldp.tile([H, NJ * ST], F32, name="ldram", tag="ld")
                for h in range(H):
                    g, r = h // 2, (h % 2) * HD
                    po = pso.tile([HD + 1, NJ * ST], F32, tag="po")
                    for b_ in range(NB):
                        if bank_slots[b_] and not bank_fast[b_]:
                            nc.vector.memset(
                                po[:, b_ * 512:(b_ + 1) * 512], 0.0)

                    def emit_av(k, runs, pt):
                        for run in runs:
                            sub = [run[0]]
                            subs = []
                            for j in run[1:]:
                                if kfirst[j] == kfirst[sub[0]]:
                                    sub.append(j)
                                else:
                                    subs.append(sub)
                                    sub = [j]
                            subs.append(sub)
                            for sub_ in subs:
                                ja, jb = sub_[0], sub_[-1]
                                b_ = ja // 4
                                fast = bank_fast[b_]
                                nc.tensor.matmul(
                                    po[0:HD + 1, ja * ST:(jb + 1) * ST],
                                    vb[:, k, h, :],
                                    pt[:, ja * ST:(jb + 1) * ST],
                                    start=(fast and k == kfirst[ja]),
                                    stop=(fast and k == bklast[b_]),
                                    skip_group_check=not fast)

                    pending = []
                    for k in range(NKT):
                        sl = slots_k[k]
                        if not sl:
                            continue
                        runs = []
                        run = [sl[0]]
                        for j in sl[1:]:
                            if j == run[-1] + 1 and j // 4 == run[0] // 4:
                                run.append(j)
                            else:
                                runs.append(run)
                                run = [j]
                        runs.append(run)
                        pt = p2s.tile([ST, NJ * ST], BF16, tag="pt", bufs=6)
                        for run in runs:
                            ja, jb = run[0], run[-1]
                            w_ = (jb + 1 - ja) * ST
                            psc = pss.tile([ST, 512], F32, tag="psc")
                            nc.tensor.matmul(
                                psc[:, 0:w_],
                                kt_t[g][r:r + HD, k * ST:(k + 1) * ST],
                                qt_t[g][r:r + HD, ja * ST:(jb + 1) * ST],
                                start=True, stop=True)
                            nc.scalar.activation(
                                pt[:, ja * ST:(jb + 1) * ST], psc[:, 0:w_],
                                mybir.ActivationFunctionType.Exp,
                                scale=1.0 / float(np.sqrt(HD)))
                        for j in sl:
                            if cls[j, k] == 2:
                                m = mixed_idx[(j, k)]
                                nc.vector.tensor_mul(
                                    pt[:, j * ST:(j + 1) * ST],
                                    pt[:, j * ST:(j + 1) * ST],
                                    mtb[:, m, :])
                        pending.append((k, runs, pt))
                        if len(pending) > 2:
                            emit_av(*pending.pop(0))
                    for args in pending:
                        emit_av(*args)
                    # unnormalized head output; 1/l = exp(-ln(l)) on ScalarE
                    nc.vector.tensor_copy(cat[g][r:r + HD, :], po[0:HD, :])
                    ltmp = p2s.tile([1, NJ * ST], F32, tag="ltmp")
                    nc.vector.tensor_copy(ltmp[:, :], po[HD:HD + 1, :])
                    rec = p2s.tile([1, NJ * ST], F32, tag="rec")
                    nc.vector.reciprocal_approx_fast(rec[:, :], ltmp[:, :])
                    nc.sync.dma_start(ldram[h:h + 1, :], rec[:, :])
                    recb = p2s.tile([ST, NJ * ST], F32, tag="recb")
                    lsrc = ldram[h]
                    nc.sync.dma_start(
                        recb[r:r + HD, :],
                        bass.AP(tensor=lsrc.tensor, offset=lsrc.offset,
                                ap=[[0, HD]] + list(lsrc.ap)))
                    nc.vector.tensor_mul(cat[g][r:r + HD, :],
                                         cat[g][r:r + HD, :],
                                         recb[r:r + HD, :])

            # ---- phase 3: fc_out ---------------------------------------
            with (
                tc.tile_pool(name="p3s", bufs=3) as p3s,
                tc.tile_pool(name="psy", bufs=2, space="PSUM") as psy,
            ):
                for jt in range(NJ):
                    py = [psy.tile([ST, 512], F32, name=f"py{n}", tag=f"py{n}")
                          for n in range(2)]
                    for c in range(NDC):
                        for n in range(2):
                            nc.tensor.matmul(
                                py[n][:, :],
                                cat[c][:, jt * ST:(jt + 1) * ST],
                                wob[:, c, n * 512:(n + 1) * 512],
                                start=(c == 0), stop=(c == NDC - 1))
                    for n in range(2):
                        ysb = p3s.tile([ST, 512], F32, tag="ysb")
                        nc.vector.tensor_add(ysb[:, :], py[n][:, :],
                                             bob[:, n * 512:(n + 1) * 512])
                        nc.sync.dma_start(
                            out_d.ap()[jt * ST:(jt + 1) * ST,
                                       n * 512:(n + 1) * 512],
                            ysb[:, :])
            late_cm.__exit__(None, None, None)

    nc.compile()
    return nc


_CACHE = {}
LAST_RESULT = None


def _get_program(mask):
    key = mask.tobytes()
    if key not in _CACHE:
        cls, mixed = _classify(mask)
        _CACHE[key] = (_build(cls, mixed, max(len(mixed), 1)), cls, mixed)
    return _CACHE[key]


def kernel(x, mask, Wq, bq, Wk, bk, Wv, bv, Wo, bo):
    x = np.ascontiguousarray(np.asarray(x, dtype=np.float32))
    mask = np.asarray(mask)
    nc, cls, mixed = _get_program(mask)

    n_maskt = max(len(mixed), 1)
    base = {
        "wq": np.ascontiguousarray(Wq, dtype=np.float32),
        "wk": np.ascontiguousarray(Wk, dtype=np.float32),
        "wv": np.ascontiguousarray(Wv, dtype=np.float32),
        "wo": np.ascontiguousarray(Wo, dtype=np.float32),
        "bq": np.ascontiguousarray(bq, dtype=np.float32),
        "bk": np.ascontiguousarray(bk, dtype=np.float32),
        "bv": np.ascontiguousarray(bv, dtype=np.float32),
        "bo": np.ascontiguousarray(bo, dtype=np.float32),
    }
    in_maps = []
    for c in range(N_CORES):
        b, h = c // 2, c % 2
        qrows = np.concatenate(
            [np.arange((2 * j + h) * ST, (2 * j + h + 1) * ST) for j in range(NJ)])
        mt = np.zeros((n_maskt, ST, ST), dtype=ml_dtypes.bfloat16)
        for i, (j, k) in enumerate(mixed):
            blk = mask[(2 * j + h) * ST:(2 * j + h + 1) * ST,
                       k * ST:(k + 1) * ST]
            mt[i] = (blk != 0).T.astype(ml_dtypes.bfloat16)
        m = dict(base)
        m["x"] = x[b]
        m["xq"] = np.ascontiguousarray(x[b][qrows])
        m["maskt"] = mt
        in_maps.append(m)

    res = run_bass_kernel_spmd(
        nc, in_maps, core_ids=list(range(N_CORES)),
        trace=os.environ.get("BASS_KERNEL_TRACE", "0") == "1")
    global LAST_RESULT
    LAST_RESULT = res

    out = np.empty((B, S, D), dtype=np.float32)
    for c in range(N_CORES):
        b, h = c // 2, c % 2
        oc = res.results[c]["out"]
        for j in range(NJ):
            out[b, (2 * j + h) * ST:(2 * j + h + 1) * ST, :] = \
                oc[j * ST:(j + 1) * ST, :]
    return out



# revision 16
# speedup vs baseline: 1.0466x; 1.0466x over previous
"""Trainium2 Bass kernel: causal multi-head attention (B=4,S=2048,D=1024,H=16).

Sharding (8 cores, no collectives): core c -> batch b=c//2, q-half h=c%2.
Each core computes all 16 heads for 8 interleaved query tiles of 128 rows
(abs q-tile t = 2*j + h for local slot j), plus full K/V for its batch,
and the full fc_out for its own query rows.  The host scatters the 8
per-core [1024,1024] outputs back into [4,2048,1024].

Device pipeline per core (all matmuls bf16, f32 accumulation), organized
to keep the tensor engine continuously busy (TRN2 PE DVFS reaches 2.4GHz
only after ~3us of uninterrupted execution) and to minimize per-ACTIVATE
fixed cost on the scalar engine (the exp bottleneck):

  A: x^T / xq^T via PE transposes (evacuations alternate scalar/vector),
     V projection for all heads (stationary x^T blocks, moving weights),
     weight-pair casts on the otherwise-idle gpsimd engine.
  B: K^T/Q^T projection for head-pair 0.
  C: per head, per k-tile: scores^T into a 2-bank PSUM tile, ONE exp
     ACTIVATE over the whole active q-range (up to 1024 wide), 0/1 mask
     multiply on mixed tiles only, out^T accumulation per 512-col group
     with ones-augmented V (row 64 = softmax denominator).  Softmax
     normalization via reciprocal + K=1-matmul partition broadcast.
     K^T/Q^T projection chunks for pair g+1 are interleaved into pair
     g's attention stream to fill tensor-engine gaps.
  D: fc_out = concat^T.T @ Wo + bo for the local query rows.

The program is specialized at build time to the mask's block structure
(skip all-zero blocks / skip masking on all-ones blocks); this is computed
from the actual mask input, so it stays correct for any mask.
"""

import os
import numpy as np
import ml_dtypes

import concourse.bass as bass
import concourse.mybir as mybir
import concourse.tile as tile
from concourse import bacc
from concourse.bass_utils import run_bass_kernel_spmd

B, S, D, H, HD = 4, 2048, 1024, 16, 64
N_CORES = 8
ST = 128               # tile edge (partition size)
NKT = S // ST          # 16 key tiles
NJ = 8                 # local query slots per core (8*128 = 1024 rows)
NDC = D // ST          # 8 contraction chunks
NG = H // 2            # 8 head pairs (2 heads packed per 128 partitions)
NSG = NKT // 4         # 4 s-groups of 512 rows
NQG = NJ // 4          # 2 q s-groups
NB = NJ // 4           # 2 x 512-col output groups of slots

F32 = mybir.dt.float32
BF16 = mybir.dt.bfloat16


def _classify(mask: np.ndarray):
    """Block structure of the mask, unioned over the two q-halves.

    Returns (cls[NJ][NKT] in {0 skip,1 full,2 mixed}, mixed list [(j,k)]).
    """
    cls = np.zeros((NJ, NKT), dtype=int)
    for j in range(NJ):
        for k in range(NKT):
            blocks = [
                mask[(2 * j + h) * ST:(2 * j + h + 1) * ST, k * ST:(k + 1) * ST]
                for h in (0, 1)
            ]
            if all((b != 0).all() for b in blocks):
                cls[j, k] = 1
            elif all((b == 0).all() for b in blocks):
                cls[j, k] = 0
            else:
                cls[j, k] = 2
        # close interior holes so every slot's computed k-range is contiguous
        nz = np.nonzero(cls[j])[0]
        if len(nz):
            for k in range(nz[0], nz[-1] + 1):
                if cls[j, k] == 0:
                    cls[j, k] = 2
    mixed = [(j, k) for j in range(NJ) for k in range(NKT) if cls[j, k] == 2]
    return cls, mixed


def _build(cls, mixed, n_maskt):
    """Build the (uniform, SPMD) Bass program for one core's shard."""
    nc = bacc.Bacc("TRN2", target_bir_lowering=False, debug=False,
                   num_devices=N_CORES)

    x_d = nc.dram_tensor("x", [S, D], F32, kind="ExternalInput")
    xq_d = nc.dram_tensor("xq", [NJ * ST, D], F32, kind="ExternalInput")
    wq_d = nc.dram_tensor("wq", [H, D, HD], F32, kind="ExternalInput")
    wk_d = nc.dram_tensor("wk", [H, D, HD], F32, kind="ExternalInput")
    wv_d = nc.dram_tensor("wv", [H, D, HD], F32, kind="ExternalInput")
    wo_d = nc.dram_tensor("wo", [D, D], F32, kind="ExternalInput")
    bq_d = nc.dram_tensor("bq", [H, HD], F32, kind="ExternalInput")
    bk_d = nc.dram_tensor("bk", [H, HD], F32, kind="ExternalInput")
    bv_d = nc.dram_tensor("bv", [H, HD], F32, kind="ExternalInput")
    bo_d = nc.dram_tensor("bo", [D], F32, kind="ExternalInput")
    mt_d = nc.dram_tensor("maskt", [n_maskt, ST, ST], BF16, kind="ExternalInput")
    out_d = nc.dram_tensor("out", [NJ * ST, D], F32, kind="ExternalOutput")

    mixed_idx = {jk: i for i, jk in enumerate(mixed)}
    slots_k = [[j for j in range(NJ) if cls[j, k]] for k in range(NKT)]
    kfirst = {}
    klast = {}
    for j in range(NJ):
        ks = [k for k in range(NKT) if cls[j, k]]
        if ks:
            kfirst[j], klast[j] = ks[0], ks[-1]

    bank_slots = [[j for j in range(4 * b_, 4 * b_ + 4) if j in kfirst]
                  for b_ in range(NB)]
    bklast = {b_: max(klast[j] for j in bank_slots[b_])
              for b_ in range(NB) if bank_slots[b_]}
    bank_fast = {b_: len({kfirst[j] for j in bank_slots[b_]}) == 1
                 for b_ in range(NB) if bank_slots[b_]}

    from concourse.masks import make_identity

    with tile.TileContext(nc) as tc:
        with (
            tc.tile_pool(name="persist", bufs=1) as pp,
        ):
            # ---- persistent SBUF tensors -------------------------------
            kt_t = [pp.tile([ST, S], BF16, name=f"ktg{g}", tag=f"ktg{g}")
                    for g in range(NG)]
            qt_t = [pp.tile([ST, NJ * ST], BF16, name=f"qtg{g}", tag=f"qtg{g}")
                    for g in range(NG)]
            vb = pp.tile([ST, NKT, H, HD + 1], BF16, name="vb", tag="vb")
            cat = [pp.tile([ST, NJ * ST], BF16, name=f"catg{g}", tag=f"catg{g}")
                   for g in range(NG)]
            wob = pp.tile([ST, NDC, D], BF16, name="wob", tag="wob")
            bob = pp.tile([ST, D], BF16, name="bob", tag="bob")
            ident = pp.tile([ST, ST], BF16, name="ident", tag="ident")
            ones1 = pp.tile([1, HD], BF16, name="ones1", tag="ones1")
            mtb = pp.tile([ST, max(n_maskt, 1), ST], BF16, name="mtb",
                          tag="mtb")

            nc.vector.memset(vb[:, :, :, HD:HD + 1], 1.0)
            nc.vector.memset(ones1[:, :], 1.0)
            make_identity(nc, ident[:, :])
            bo_ap = bo_d.ap()
            nc.gpsimd.dma_start(
                bob[:, :],
                bass.AP(tensor=bo_ap.tensor, offset=bo_ap.offset,
                        ap=[[0, ST]] + list(bo_ap.ap)))
            nc.sync.dma_start(mtb[:, :, :],
                              mt_d.ap().rearrange("m p f -> p m f"))

            def load_bias_pair(pool, bias_d, name):
                # [128, NG] f32: partition = (h%2)*64+e, column = pair idx
                t = pool.tile([ST, NG], F32, name=name, tag=name, bufs=1)
                src = bias_d.ap()
                nc.scalar.dma_start(
                    t[:, :],
                    bass.AP(tensor=src.tensor, offset=src.offset,
                            ap=[[1, ST], [ST, NG]]))
                return t

            bkp = load_bias_pair(pp, bk_d, "bkp")
            bqp = load_bias_pair(pp, bq_d, "bqp")

            # x^T tiles (live until the last K/Q projection)
            xtp_cm = tc.tile_pool(name="xtp", bufs=1, side="right")
            xtp = xtp_cm.__enter__()
            xt = {}
            for c in range(NDC):
                for sg in range(NSG):
                    xt[c, sg] = xtp.tile([ST, 512], BF16,
                                         name=f"xt{c}_{sg}", tag=f"xt{c}_{sg}")
            xqt = {}
            for c in range(NDC):
                for sg in range(NQG):
                    xqt[c, sg] = xtp.tile([ST, 512], BF16,
                                          name=f"xqt{c}_{sg}",
                                          tag=f"xqt{c}_{sg}")

            # ---- phase A: x^T, xq^T, V ---------------------------------
            with (
                tc.tile_pool(name="p1a", bufs=2) as p1a,
                tc.tile_pool(name="pv", bufs=1) as pv,
                tc.tile_pool(name="ppst", bufs=4, space="PSUM") as ppst,
                tc.tile_pool(name="ppsv", bufs=2, space="PSUM") as ppsv,
            ):
                # V weights: staged f32 DMA, cast to bf16 on gpsimd
                wvb = pv.tile([ST, NDC, H, HD], BF16, name="wvb", tag="wvb",
                              bufs=1)
                bvf = pv.tile([ST, H, HD], F32, name="bvf", tag="bvf", bufs=1)
                srcv = bv_d.ap()
                nc.sync.dma_start(
                    bvf[:, :, :],
                    bass.AP(tensor=srcv.tensor, offset=srcv.offset,
                            ap=[[0, ST]] + list(srcv.ap)))
                for hh in range(H):
                    src = wv_d.ap()[hh].rearrange("(c p) e -> p c e", p=ST)
                    wstg = p1a.tile([ST, NDC, HD], F32, tag="wstg")
                    nc.scalar.dma_start(wstg[:, :, :], src)
                    nc.gpsimd.tensor_copy(wvb[:, :, hh, :], wstg[:, :, :])

                evac_i = 0
                for st in range(NKT):
                    sg, so = st // 4, (st % 4) * ST
                    xf = p1a.tile([ST, D], F32, tag="xf")
                    nc.sync.dma_start(
                        xf[:, :], x_d.ap()[st * ST:(st + 1) * ST, :])
                    xb = p1a.tile([ST, D], BF16, tag="xb")
                    nc.vector.tensor_copy(xb[:, :], xf[:, :])
                    for c in range(NDC):
                        pst = ppst.tile([ST, ST], BF16, tag="pst")
                        nc.tensor.transpose(
                            pst[:, :], xb[:, c * ST:(c + 1) * ST], ident[:, :])
                        eng = nc.scalar if evac_i % 2 == 0 else nc.vector
                        if eng is nc.scalar:
                            nc.scalar.copy(xt[c, sg][:, so:so + ST], pst[:, :])
                        else:
                            nc.vector.tensor_copy(xt[c, sg][:, so:so + ST],
                                                  pst[:, :])
                        evac_i += 1
                    # V projection for this s-tile (all 16 heads)
                    psv = ppsv.tile([ST, H * HD], F32, tag="psv")
                    for c in range(NDC):
                        for n in range(2):
                            nc.tensor.matmul(
                                psv[:, n * 512:(n + 1) * 512],
                                xt[c, sg][:, so:so + ST],
                                wvb[:, c, 8 * n:8 * n + 8, :],
                                start=(c == 0), stop=(c == NDC - 1))
                    nc.vector.tensor_add(
                        vb[:, st, :, 0:HD],
                        psv[:, :].rearrange("p (h e) -> p h e", h=H),
                        bvf[:, :, :])

                for jl in range(NJ):
                    sg, so = jl // 4, (jl % 4) * ST
                    xf = p1a.tile([ST, D], F32, tag="xf")
                    nc.sync.dma_start(xf[:, :],
                                      xq_d.ap()[jl * ST:(jl + 1) * ST, :])
                    xb = p1a.tile([ST, D], BF16, tag="xb")
                    nc.vector.tensor_copy(xb[:, :], xf[:, :])
                    for c in range(NDC):
                        pst = ppst.tile([ST, ST], BF16, tag="pst")
                        nc.tensor.transpose(
                            pst[:, :], xb[:, c * ST:(c + 1) * ST], ident[:, :])
                        eng_scalar = evac_i % 2 == 0
                        if eng_scalar:
                            nc.scalar.copy(xqt[c, sg][:, so:so + ST],
                                           pst[:, :])
                        else:
                            nc.vector.tensor_copy(xqt[c, sg][:, so:so + ST],
                                                  pst[:, :])
                        evac_i += 1

            # ---- phases B/C/D: projections + attention + fc_out --------
            # PSUM budget: psc 2x2 banks + po 3x1 banks + psk 1x1 = 8.
            wp_cm = tc.tile_pool(name="wpair", bufs=2)
            wp = wp_cm.__enter__()
            p2s_cm = tc.tile_pool(name="p2s", bufs=2)
            p2s = p2s_cm.__enter__()
            pt_cm = tc.tile_pool(name="ptp", bufs=3)
            ptp = pt_cm.__enter__()
            psc_cm = tc.tile_pool(name="psc", bufs=2, space="PSUM")
            pscp = psc_cm.__enter__()
            po_cm = tc.tile_pool(name="po", bufs=3, space="PSUM")
            pop = po_cm.__enter__()
            psk_cm = tc.tile_pool(name="psk", bufs=1, space="PSUM")
            pskp = psk_cm.__enter__()

            def stage_pair_weights(w_d, g, tag):
                # DMA both heads of pair g (f32), cast to the [128, NDC, 128]
                # stationary-pair layout on gpsimd.
                src = w_d.ap()[2 * g:2 * g + 2].rearrange(
                    "h (c p) e -> p h c e", p=ST)
                wstg2 = p2s.tile([ST, 2, NDC, HD], F32, tag="wstg2")
                nc.scalar.dma_start(wstg2[:, :, :, :], src)
                wpr = wp.tile([ST, NDC, ST], BF16, name=f"{tag}{g}", tag=tag)
                for h2 in range(2):
                    nc.gpsimd.tensor_copy(
                        wpr[:, :, h2 * HD:(h2 + 1) * HD],
                        wstg2[:, h2, :, :])
                return wpr

            def k_proj_block(wpr, g, sg):
                psk = pskp.tile([ST, 512], F32, tag="psk")
                for c in range(NDC):
                    nc.tensor.matmul(
                        psk[:, :], wpr[:, c, :], xt[c, sg][:, :],
                        start=(c == 0), stop=(c == NDC - 1))
                nc.vector.tensor_scalar(
                    kt_t[g][:, sg * 512:(sg + 1) * 512],
                    psk[:, :], bkp[:, g:g + 1], None,
                    mybir.AluOpType.add)

            def q_proj_block(wpr, g, sg):
                psk = pskp.tile([ST, 512], F32, tag="psk")
                for c in range(NDC):
                    nc.tensor.matmul(
                        psk[:, :], wpr[:, c, :], xqt[c, sg][:, :],
                        start=(c == 0), stop=(c == NDC - 1))
                nc.vector.tensor_scalar(
                    qt_t[g][:, sg * 512:(sg + 1) * 512],
                    psk[:, :], bqp[:, g:g + 1], None,
                    mybir.AluOpType.add)

            def proj_chunks_for_pair(g):
                # closures emitting one tensor-engine chunk each
                wk_pr = stage_pair_weights(wk_d, g, "wkpr")
                wq_pr = stage_pair_weights(wq_d, g, "wqpr")
                chunks = []
                for sg in range(NSG):
                    chunks.append(lambda sg=sg: k_proj_block(wk_pr, g, sg))
                for sg in range(NQG):
                    chunks.append(lambda sg=sg: q_proj_block(wq_pr, g, sg))
                return chunks

            # pair 0 projections up front
            for ch in proj_chunks_for_pair(0):
                ch()

            def attention_head(g, h, pending_chunks):
                """Emit attention for head h (pair g). pending_chunks is a
                list of proj closures for pair g+1, drained ~evenly."""
                r = (h % 2) * HD
                po = {}
                for b_ in range(NB):
                    if bank_slots[b_]:
                        po[b_] = pop.tile([HD + 1, 512], F32, tag="po",
                                          name=f"po{h}_{b_}")
                        if not bank_fast[b_]:
                            nc.vector.memset(po[b_][:, :], 0.0)

                active_ks = [k for k in range(NKT) if slots_k[k]]
                n_ks = len(active_ks)
                drain_every = max(1, n_ks // max(len(pending_chunks), 1) + 1) \
                    if pending_chunks else 0

                def emit_av(k, runs, pt):
                    for run in runs:
                        sub = [run[0]]
                        subs = []
                        for j in run[1:]:
                            if kfirst[j] == kfirst[sub[0]]:
                                sub.append(j)
                            else:
                                subs.append(sub)
                                sub = [j]
                        subs.append(sub)
                        for sub_ in subs:
                            ja, jb = sub_[0], sub_[-1]
                            b_ = ja // 4
                            fast = bank_fast[b_]
                            nc.tensor.matmul(
                                po[b_][0:HD + 1,
                                       (ja - 4 * b_) * ST:(jb + 1 - 4 * b_) * ST],
                                vb[:, k, h, :],
                                pt[:, ja * ST:(jb + 1) * ST],
                                start=(fast and k == kfirst[ja]),
                                stop=(fast and k == bklast[b_]),
                                skip_group_check=not fast)

                pending = []
                for ki, k in enumerate(active_ks):
                    sl = slots_k[k]
                    runs = []
                    run = [sl[0]]
                    for j in sl[1:]:
                        if j == run[-1] + 1 and j // 4 == run[0] // 4:
                            run.append(j)
                        else:
                            runs.append(run)
                            run = [j]
                    runs.append(run)
                    psc = pscp.tile([ST, NJ * ST], F32, tag="psc")
                    for run in runs:
                        ja, jb = run[0], run[-1]
                        nc.tensor.matmul(
                            psc[:, ja * ST:(jb + 1) * ST],
                            kt_t[g][r:r + HD, k * ST:(k + 1) * ST],
                            qt_t[g][r:r + HD, ja * ST:(jb + 1) * ST],
                            start=True, stop=True)
                    # one exp over the bounding active range of this k-tile
                    jaT, jbT = sl[0], sl[-1]
                    pt = ptp.tile([ST, NJ * ST], BF16, tag="pt")
                    nc.scalar.activation(
                        pt[:, jaT * ST:(jbT + 1) * ST],
                        psc[:, jaT * ST:(jbT + 1) * ST],
                        mybir.ActivationFunctionType.Exp,
                        scale=1.0 / float(np.sqrt(HD)))
                    for j in sl:
                        if cls[j, k] == 2:
                            m = mixed_idx[(j, k)]
                            nc.vector.tensor_mul(
                                pt[:, j * ST:(j + 1) * ST],
                                pt[:, j * ST:(j + 1) * ST],
                                mtb[:, m, :])
                    pending.append((k, runs, pt))
                    if len(pending) > 1:
                        emit_av(*pending.pop(0))
                    if pending_chunks and drain_every and \
                            ki % drain_every == drain_every - 1:
                        pending_chunks.pop(0)()
                for args in pending:
                    emit_av(*args)
                while pending_chunks:
                    pending_chunks.pop(0)()

                # normalization: 1/denominator broadcast via K=1 matmul
                for b_ in range(NB):
                    if not bank_slots[b_]:
                        continue
                    ltmp = p2s.tile([1, 512], F32, tag="ltmp")
                    nc.vector.tensor_copy(ltmp[:, :], po[b_][HD:HD + 1, :])
                    rec = p2s.tile([1, 512], F32, tag="rec")
                    nc.vector.reciprocal_approx_fast(rec[:, :], ltmp[:, :])
                    rec16 = p2s.tile([1, 512], BF16, tag="rec16")
                    nc.vector.tensor_copy(rec16[:, :], rec[:, :])
                    recps = pskp.tile([HD, 512], F32, tag="psk")
                    nc.tensor.matmul(recps[:, :], ones1[:, :], rec16[:, :],
                                     start=True, stop=True)
                    cslice = cat[g][r:r + HD, 4 * b_ * ST:(4 * b_ + 4) * ST]
                    nc.vector.tensor_copy(cslice, po[b_][0:HD, :])
                    nc.vector.tensor_mul(cslice, cslice, recps[:, :])

            for g in range(NG):
                chunks = proj_chunks_for_pair(g + 1) if g + 1 < NG else []
                # split interleaved proj chunks between the two heads
                half = (len(chunks) + 1) // 2
                attention_head(g, 2 * g, chunks[:half])
                attention_head(g, 2 * g + 1, chunks[half:])

            # Wo load (staged during late attention)
            for c in range(NDC):
                for n in range(2):
                    wstg3 = p2s.tile([ST, 512], F32, tag="wstg3")
                    nc.sync.dma_start(
                        wstg3[:, :],
                        wo_d.ap()[c * ST:(c + 1) * ST,
                                  n * 512:(n + 1) * 512])
                    nc.gpsimd.tensor_copy(wob[:, c, n * 512:(n + 1) * 512],
                                          wstg3[:, :])

            psk_cm.__exit__(None, None, None)
            po_cm.__exit__(None, None, None)
            psc_cm.__exit__(None, None, None)
            pt_cm.__exit__(None, None, None)
            p2s_cm.__exit__(None, None, None)
            wp_cm.__exit__(None, None, None)

            # ---- phase D: fc_out ---------------------------------------
            with (
                tc.tile_pool(name="p3s", bufs=3) as p3s,
                tc.tile_pool(name="psy", bufs=4, space="PSUM") as psy,
            ):
                for jt in range(NJ):
                    py = [psy.tile([ST, 512], F32, tag="py",
                                   name=f"py{jt}_{n}") for n in range(2)]
                    for c in range(NDC):
                        for n in range(2):
                            nc.tensor.matmul(
                                py[n][:, :],
                                cat[c][:, jt * ST:(jt + 1) * ST],
                                wob[:, c, n * 512:(n + 1) * 512],
                                start=(c == 0), stop=(c == NDC - 1))
                    for n in range(2):
                        ysb = p3s.tile([ST, 512], F32, tag="ysb")
                        nc.vector.tensor_add(ysb[:, :], py[n][:, :],
                                             bob[:, n * 512:(n + 1) * 512])
                        nc.sync.dma_start(
                            out_d.ap()[jt * ST:(jt + 1) * ST,
                                       n * 512:(n + 1) * 512],
                            ysb[:, :])

            xtp_cm.__exit__(None, None, None)

    nc.compile()
    return nc


_CACHE = {}
LAST_RESULT = None


def _get_program(mask):
    key = mask.tobytes()
    if key not in _CACHE:
        cls, mixed = _classify(mask)
        _CACHE[key] = (_build(cls, mixed, max(len(mixed), 1)), cls, mixed)
    return _CACHE[key]


def kernel(x, mask, Wq, bq, Wk, bk, Wv, bv, Wo, bo):
    x = np.ascontiguousarray(np.asarray(x, dtype=np.float32))
    mask = np.asarray(mask)
    nc, cls, mixed = _get_program(mask)

    n_maskt = max(len(mixed), 1)
    base = {
        "wq": np.ascontiguousarray(Wq, dtype=np.float32),
        "wk": np.ascontiguousarray(Wk, dtype=np.float32),
        "wv": np.ascontiguousarray(Wv, dtype=np.float32),
        "wo": np.ascontiguousarray(Wo, dtype=np.float32),
        "bq": np.ascontiguousarray(bq, dtype=np.float32),
        "bk": np.ascontiguousarray(bk, dtype=np.float32),
        "bv": np.ascontiguousarray(bv, dtype=np.float32),
        "bo": np.ascontiguousarray(bo, dtype=np.float32),
    }
    in_maps = []
    for c in range(N_CORES):
        b, h = c // 2, c % 2
        qrows = np.concatenate(
            [np.arange((2 * j + h) * ST, (2 * j + h + 1) * ST) for j in range(NJ)])
        mt = np.zeros((n_maskt, ST, ST), dtype=ml_dtypes.bfloat16)
        for i, (j, k) in enumerate(mixed):
            blk = mask[(2 * j + h) * ST:(2 * j + h + 1) * ST,
                       k * ST:(k + 1) * ST]
            mt[i] = (blk != 0).T.astype(ml_dtypes.bfloat16)
        m = dict(base)
        m["x"] = x[b]
        m["xq"] = np.ascontiguousarray(x[b][qrows])
        m["maskt"] = mt
        in_maps.append(m)

    res = run_bass_kernel_spmd(
        nc, in_maps, core_ids=list(range(N_CORES)),
        trace=os.environ.get("BASS_KERNEL_TRACE", "0") == "1")
    global LAST_RESULT
    LAST_RESULT = res

    out = np.empty((B, S, D), dtype=np.float32)
    for c in range(N_CORES):
        b, h = c // 2, c % 2
        oc = res.results[c]["out"]
        for j in range(NJ):
            out[b, (2 * j + h) * ST:(2 * j + h + 1) * ST, :] = \
                oc[j * ST:(j + 1) * ST, :]
    return out


# revision 20
# speedup vs baseline: 1.1054x; 1.0562x over previous
"""Trainium2 Bass kernel: causal multi-head attention (B=4,S=2048,D=1024,H=16).

Sharding (8 cores, no collectives): core c -> batch b=c//2, q-half h=c%2.
Each core computes all 16 heads for 8 interleaved query tiles of 128 rows
(abs q-tile t = 2*j + h for local slot j), plus full K/V for its batch,
and the full fc_out for its own query rows.  The host scatters the 8
per-core [1024,1024] outputs back into [4,2048,1024].

Device pipeline per core (all matmuls bf16, f32 accumulation), organized
to keep the tensor engine continuously busy (TRN2 PE DVFS reaches 2.4GHz
only after ~3us of uninterrupted execution) and to minimize per-ACTIVATE
fixed cost on the scalar engine (the exp bottleneck):

  A: x^T / xq^T via PE transposes (evacuations alternate scalar/vector),
     V projection for all heads (stationary x^T blocks, moving weights),
     weight-pair casts on the otherwise-idle gpsimd engine.
  B: K^T/Q^T projection for head-pair 0.
  C: per head, per k-tile: scores^T into a 2-bank PSUM tile, ONE exp
     ACTIVATE over the whole active q-range (up to 1024 wide), 0/1 mask
     multiply on mixed tiles only, out^T accumulation per 512-col group
     with ones-augmented V (row 64 = softmax denominator).  Softmax
     normalization via reciprocal + K=1-matmul partition broadcast.
     K^T/Q^T projection chunks for pair g+1 are interleaved into pair
     g's attention stream to fill tensor-engine gaps.
  D: fc_out = concat^T.T @ Wo + bo for the local query rows.

The program is specialized at build time to the mask's block structure
(skip all-zero blocks / skip masking on all-ones blocks); this is computed
from the actual mask input, so it stays correct for any mask.
"""

import os
import numpy as np
import ml_dtypes

import concourse.bass as bass
import concourse.mybir as mybir
import concourse.tile as tile
from concourse import bacc
from concourse.bass_utils import run_bass_kernel_spmd

B, S, D, H, HD = 4, 2048, 1024, 16, 64
N_CORES = 8
ST = 128               # tile edge (partition size)
NKT = S // ST          # 16 key tiles
NJ = 8                 # local query slots per core (8*128 = 1024 rows)
NDC = D // ST          # 8 contraction chunks
NG = H // 2            # 8 head pairs (2 heads packed per 128 partitions)
NSG = NKT // 4         # 4 s-groups of 512 rows
NQG = NJ // 4          # 2 q s-groups
NB = NJ // 4           # 2 x 512-col output groups of slots

F32 = mybir.dt.float32
BF16 = mybir.dt.bfloat16


def _classify(mask: np.ndarray):
    """Block structure of the mask, unioned over the two q-halves.

    Returns (cls[NJ][NKT] in {0 skip,1 full,2 mixed}, mixed list [(j,k)]).
    """
    cls = np.zeros((NJ, NKT), dtype=int)
    for j in range(NJ):
        for k in range(NKT):
            blocks = [
                mask[(2 * j + h) * ST:(2 * j + h + 1) * ST, k * ST:(k + 1) * ST]
                for h in (0, 1)
            ]
            if all((b != 0).all() for b in blocks):
                cls[j, k] = 1
            elif all((b == 0).all() for b in blocks):
                cls[j, k] = 0
            else:
                cls[j, k] = 2
        # close interior holes so every slot's computed k-range is contiguous
        nz = np.nonzero(cls[j])[0]
        if len(nz):
            for k in range(nz[0], nz[-1] + 1):
                if cls[j, k] == 0:
                    cls[j, k] = 2
    mixed = [(j, k) for j in range(NJ) for k in range(NKT) if cls[j, k] == 2]
    # dedup mixed tiles by their (h=0, h=1) block content pair: the causal
    # mask yields only 2 distinct patterns, saving SBUF and DMA
    dedup = {}
    midx = {}
    for (j, k) in mixed:
        key = tuple(
            mask[(2 * j + h) * ST:(2 * j + h + 1) * ST,
                 k * ST:(k + 1) * ST].tobytes() for h in (0, 1))
        if key not in dedup:
            dedup[key] = len(dedup)
        midx[(j, k)] = dedup[key]
    return cls, mixed, midx, max(len(dedup), 1)


def _build(cls, mixed, mixed_idx, n_maskt):
    """Build the (uniform, SPMD) Bass program for one core's shard."""
    nc = bacc.Bacc("TRN2", target_bir_lowering=False, debug=False,
                   num_devices=N_CORES)

    x_d = nc.dram_tensor("x", [S, D], F32, kind="ExternalInput")
    xq_d = nc.dram_tensor("xq", [NJ * ST, D], F32, kind="ExternalInput")
    wq_d = nc.dram_tensor("wq", [H, D, HD], F32, kind="ExternalInput")
    wk_d = nc.dram_tensor("wk", [H, D, HD], F32, kind="ExternalInput")
    wv_d = nc.dram_tensor("wv", [H, D, HD], F32, kind="ExternalInput")
    wo_d = nc.dram_tensor("wo", [D, D], F32, kind="ExternalInput")
    bq_d = nc.dram_tensor("bq", [H, HD], F32, kind="ExternalInput")
    bk_d = nc.dram_tensor("bk", [H, HD], F32, kind="ExternalInput")
    bv_d = nc.dram_tensor("bv", [H, HD], F32, kind="ExternalInput")
    bo_d = nc.dram_tensor("bo", [D], F32, kind="ExternalInput")
    mt_d = nc.dram_tensor("maskt", [n_maskt, ST, ST], BF16, kind="ExternalInput")
    out_d = nc.dram_tensor("out", [NJ * ST, D], F32, kind="ExternalOutput")

    slots_k = [[j for j in range(NJ) if cls[j, k]] for k in range(NKT)]
    kfirst = {}
    klast = {}
    for j in range(NJ):
        ks = [k for k in range(NKT) if cls[j, k]]
        if ks:
            kfirst[j], klast[j] = ks[0], ks[-1]

    bank_slots = [[j for j in range(4 * b_, 4 * b_ + 4) if j in kfirst]
                  for b_ in range(NB)]
    bklast = {b_: max(klast[j] for j in bank_slots[b_])
              for b_ in range(NB) if bank_slots[b_]}
    bank_fast = {b_: len({kfirst[j] for j in bank_slots[b_]}) == 1
                 for b_ in range(NB) if bank_slots[b_]}

    from concourse.masks import make_identity

    with tile.TileContext(nc) as tc:
        with (
            tc.tile_pool(name="persist", bufs=1) as pp,
        ):
            # ---- persistent SBUF tensors -------------------------------
            kt_t = [pp.tile([ST, S], BF16, name=f"ktg{g}", tag=f"ktg{g}")
                    for g in range(NG)]
            qt_t = [pp.tile([ST, NJ * ST], BF16, name=f"qtg{g}", tag=f"qtg{g}")
                    for g in range(NG)]
            vb = pp.tile([ST, NKT, H, HD + 1], BF16, name="vb", tag="vb")
            cat = [pp.tile([ST, NJ * ST], BF16, name=f"catg{g}", tag=f"catg{g}")
                   for g in range(NG)]
            wob = pp.tile([ST, NDC, D], BF16, name="wob", tag="wob")
            bob = pp.tile([ST, D], BF16, name="bob", tag="bob")
            ident = pp.tile([ST, ST], BF16, name="ident", tag="ident")
            ones1 = pp.tile([1, HD], BF16, name="ones1", tag="ones1")
            mtb = pp.tile([ST, max(n_maskt, 1), ST], BF16, name="mtb",
                          tag="mtb")

            nc.vector.memset(vb[:, :, :, HD:HD + 1], 1.0)
            nc.vector.memset(ones1[:, :], 1.0)
            make_identity(nc, ident[:, :])
            bo_ap = bo_d.ap()
            nc.gpsimd.dma_start(
                bob[:, :],
                bass.AP(tensor=bo_ap.tensor, offset=bo_ap.offset,
                        ap=[[0, ST]] + list(bo_ap.ap)))
            nc.sync.dma_start(mtb[:, :, :],
                              mt_d.ap().rearrange("m p f -> p m f"))

            def load_bias_pair(pool, bias_d, name):
                # [128, NG] f32: partition = (h%2)*64+e, column = pair idx
                t = pool.tile([ST, NG], F32, name=name, tag=name, bufs=1)
                src = bias_d.ap()
                nc.scalar.dma_start(
                    t[:, :],
                    bass.AP(tensor=src.tensor, offset=src.offset,
                            ap=[[1, ST], [ST, NG]]))
                return t

            bkp = load_bias_pair(pp, bk_d, "bkp")
            bqp = load_bias_pair(pp, bq_d, "bqp")

            # x^T tiles (live until the last K/Q projection)
            xtp_cm = tc.tile_pool(name="xtp", bufs=1, side="right")
            xtp = xtp_cm.__enter__()
            xt = {}
            for c in range(NDC):
                for sg in range(NSG):
                    xt[c, sg] = xtp.tile([ST, 512], BF16,
                                         name=f"xt{c}_{sg}", tag=f"xt{c}_{sg}")
            xqt = {}
            for c in range(NDC):
                for sg in range(NQG):
                    xqt[c, sg] = xtp.tile([ST, 512], BF16,
                                          name=f"xqt{c}_{sg}",
                                          tag=f"xqt{c}_{sg}")

            # ---- phase A: x^T, xq^T, V ---------------------------------
            with (
                tc.tile_pool(name="p1a", bufs=2) as p1a,
                tc.tile_pool(name="pv", bufs=1) as pv,
                tc.tile_pool(name="ppst", bufs=4, space="PSUM") as ppst,
                tc.tile_pool(name="ppsv", bufs=2, space="PSUM") as ppsv,
            ):
                # V weights: staged f32 DMA, cast to bf16 on gpsimd
                wvb = pv.tile([ST, NDC, H, HD], BF16, name="wvb", tag="wvb",
                              bufs=1)
                bvf = pv.tile([ST, H, HD], F32, name="bvf", tag="bvf", bufs=1)
                srcv = bv_d.ap()
                nc.sync.dma_start(
                    bvf[:, :, :],
                    bass.AP(tensor=srcv.tensor, offset=srcv.offset,
                            ap=[[0, ST]] + list(srcv.ap)))
                for hh in range(H):
                    src = wv_d.ap()[hh].rearrange("(c p) e -> p c e", p=ST)
                    wstg = p1a.tile([ST, NDC, HD], F32, tag="wstg")
                    nc.scalar.dma_start(wstg[:, :, :], src)
                    nc.vector.tensor_copy(wvb[:, :, hh, :], wstg[:, :, :])

                evac_i = 0
                for st in range(NKT):
                    sg, so = st // 4, (st % 4) * ST
                    xf = p1a.tile([ST, D], F32, tag="xf")
                    qeng = nc.sync if st % 2 == 0 else nc.scalar
                    qeng.dma_start(
                        xf[:, :], x_d.ap()[st * ST:(st + 1) * ST, :])
                    xb = p1a.tile([ST, D], BF16, tag="xb")
                    nc.vector.tensor_copy(xb[:, :], xf[:, :])
                    for c in range(NDC):
                        pst = ppst.tile([ST, ST], BF16, tag="pst")
                        nc.tensor.transpose(
                            pst[:, :], xb[:, c * ST:(c + 1) * ST], ident[:, :])
                        eng = nc.scalar if evac_i % 2 == 0 else nc.vector
                        if eng is nc.scalar:
                            nc.scalar.copy(xt[c, sg][:, so:so + ST], pst[:, :])
                        else:
                            nc.vector.tensor_copy(xt[c, sg][:, so:so + ST],
                                                  pst[:, :])
                        evac_i += 1
                    # V projection for this s-tile (all 16 heads)
                    psv = ppsv.tile([ST, H * HD], F32, tag="psv")
                    for c in range(NDC):
                        for n in range(2):
                            nc.tensor.matmul(
                                psv[:, n * 512:(n + 1) * 512],
                                xt[c, sg][:, so:so + ST],
                                wvb[:, c, 8 * n:8 * n + 8, :],
                                start=(c == 0), stop=(c == NDC - 1))
                    nc.vector.tensor_add(
                        vb[:, st, :, 0:HD],
                        psv[:, :].rearrange("p (h e) -> p h e", h=H),
                        bvf[:, :, :])

                for jl in range(NJ):
                    sg, so = jl // 4, (jl % 4) * ST
                    xf = p1a.tile([ST, D], F32, tag="xf")
                    qeng = nc.sync if jl % 2 == 0 else nc.scalar
                    qeng.dma_start(xf[:, :],
                                   xq_d.ap()[jl * ST:(jl + 1) * ST, :])
                    xb = p1a.tile([ST, D], BF16, tag="xb")
                    nc.vector.tensor_copy(xb[:, :], xf[:, :])
                    for c in range(NDC):
                        pst = ppst.tile([ST, ST], BF16, tag="pst")
                        nc.tensor.transpose(
                            pst[:, :], xb[:, c * ST:(c + 1) * ST], ident[:, :])
                        eng_scalar = evac_i % 2 == 0
                        if eng_scalar:
                            nc.scalar.copy(xqt[c, sg][:, so:so + ST],
                                           pst[:, :])
                        else:
                            nc.vector.tensor_copy(xqt[c, sg][:, so:so + ST],
                                                  pst[:, :])
                        evac_i += 1

            # ---- phases B/C/D: projections + attention + fc_out --------
            # PSUM budget: psc 2x2 banks + po 3x1 banks + psk 1x1 = 8.
            wp_cm = tc.tile_pool(name="wpair", bufs=2)
            wp = wp_cm.__enter__()
            p2s_cm = tc.tile_pool(name="p2s", bufs=2)
            p2s = p2s_cm.__enter__()
            pt_cm = tc.tile_pool(name="ptp", bufs=2)
            ptp = pt_cm.__enter__()
            psc_cm = tc.tile_pool(name="psc", bufs=1, space="PSUM")
            pscp = psc_cm.__enter__()
            po_cm = tc.tile_pool(name="po", bufs=3, space="PSUM")
            pop = po_cm.__enter__()
            psk_cm = tc.tile_pool(name="psk", bufs=1, space="PSUM")
            pskp = psk_cm.__enter__()

            def stage_pair_weights(w_d, g, tag):
                # DMA both heads of pair g (f32), cast to the [128, NDC, 128]
                # stationary-pair layout on gpsimd.
                src = w_d.ap()[2 * g:2 * g + 2].rearrange(
                    "h (c p) e -> p h c e", p=ST)
                wstg2 = p2s.tile([ST, 2, NDC, HD], F32, tag="wstg2")
                nc.scalar.dma_start(wstg2[:, :, :, :], src)
                wpr = wp.tile([ST, NDC, ST], BF16, name=f"{tag}{g}", tag=tag)
                for h2 in range(2):
                    nc.vector.tensor_copy(
                        wpr[:, :, h2 * HD:(h2 + 1) * HD],
                        wstg2[:, h2, :, :])
                return wpr

            def k_proj_block(wpr, g, sg):
                psk = pskp.tile([ST, 512], F32, tag="psk")
                for c in range(NDC):
                    nc.tensor.matmul(
                        psk[:, :], wpr[:, c, :], xt[c, sg][:, :],
                        start=(c == 0), stop=(c == NDC - 1))
                nc.vector.tensor_scalar(
                    kt_t[g][:, sg * 512:(sg + 1) * 512],
                    psk[:, :], bkp[:, g:g + 1], None,
                    mybir.AluOpType.add)

            def q_proj_block(wpr, g, sg):
                psk = pskp.tile([ST, 512], F32, tag="psk")
                for c in range(NDC):
                    nc.tensor.matmul(
                        psk[:, :], wpr[:, c, :], xqt[c, sg][:, :],
                        start=(c == 0), stop=(c == NDC - 1))
                nc.vector.tensor_scalar(
                    qt_t[g][:, sg * 512:(sg + 1) * 512],
                    psk[:, :], bqp[:, g:g + 1], None,
                    mybir.AluOpType.add)

            def proj_chunks_for_pair(g):
                # closures emitting one tensor-engine chunk each
                wk_pr = stage_pair_weights(wk_d, g, "wkpr")
                wq_pr = stage_pair_weights(wq_d, g, "wqpr")
                chunks = []
                for sg in range(NSG):
                    chunks.append(lambda sg=sg: k_proj_block(wk_pr, g, sg))
                for sg in range(NQG):
                    chunks.append(lambda sg=sg: q_proj_block(wq_pr, g, sg))
                return chunks

            # pair 0 projections up front
            for ch in proj_chunks_for_pair(0):
                ch()

            def attention_head(g, h, pending_chunks):
                """Emit attention for head h (pair g). pending_chunks is a
                list of proj closures for pair g+1, drained ~evenly."""
                r = (h % 2) * HD
                po = {}
                for b_ in range(NB):
                    if bank_slots[b_]:
                        po[b_] = pop.tile([HD + 1, 512], F32, tag="po",
                                          name=f"po{h}_{b_}")
                        if not bank_fast[b_]:
                            nc.vector.memset(po[b_][:, :], 0.0)

                active_ks = [k for k in range(NKT) if slots_k[k]]
                n_ep = (len(active_ks) + 1) // 2
                drain_every = max(1, n_ep // max(len(pending_chunks), 1)) \
                    if pending_chunks else 0

                def emit_av(infos, pt):
                    for k, runs, par in infos:
                        for run in runs:
                            sub = [run[0]]
                            subs = []
                            for j in run[1:]:
                                if kfirst[j] == kfirst[sub[0]]:
                                    sub.append(j)
                                else:
                                    subs.append(sub)
                                    sub = [j]
                            subs.append(sub)
                            for sub_ in subs:
                                ja, jb = sub_[0], sub_[-1]
                                b_ = ja // 4
                                fast = bank_fast[b_]
                                nc.tensor.matmul(
                                    po[b_][0:HD + 1,
                                           (ja - 4 * b_) * ST:
                                           (jb + 1 - 4 * b_) * ST],
                                    vb[:, k, h, :],
                                    pt[:, par, ja * ST:(jb + 1) * ST],
                                    start=(fast and k == kfirst[ja]),
                                    stop=(fast and k == bklast[b_]),
                                    skip_group_check=not fast)

                def runs_of(sl):
                    runs = []
                    run = [sl[0]]
                    for j in sl[1:]:
                        if j == run[-1] + 1 and j // 4 == run[0] // 4:
                            run.append(j)
                        else:
                            runs.append(run)
                            run = [j]
                    runs.append(run)
                    return runs

                epochs = [active_ks[i:i + 2]
                          for i in range(0, len(active_ks), 2)]
                pending = []
                for ei, eks in enumerate(epochs):
                    psc = pscp.tile([ST, 2, NJ * ST], F32, tag="psc")
                    infos = []
                    for par, k in enumerate(eks):
                        runs = runs_of(slots_k[k])
                        for run in runs:
                            ja, jb = run[0], run[-1]
                            nc.tensor.matmul(
                                psc[:, par, ja * ST:(jb + 1) * ST],
                                kt_t[g][r:r + HD, k * ST:(k + 1) * ST],
                                qt_t[g][r:r + HD, ja * ST:(jb + 1) * ST],
                                start=True, stop=True)
                        infos.append((k, runs, par))
                    # one exp over the union active range of the k-pair
                    jaT = min(slots_k[k][0] for k in eks)
                    jbT = max(slots_k[k][-1] for k in eks)
                    npar = len(eks)
                    pt = ptp.tile([ST, 2, NJ * ST], BF16, tag="pt")
                    nc.scalar.activation(
                        pt[:, 0:npar, jaT * ST:(jbT + 1) * ST],
                        psc[:, 0:npar, jaT * ST:(jbT + 1) * ST],
                        mybir.ActivationFunctionType.Exp,
                        scale=1.0 / float(np.sqrt(HD)))
                    for k, runs, par in infos:
                        for j in slots_k[k]:
                            if cls[j, k] == 2:
                                m = mixed_idx[(j, k)]
                                nc.vector.tensor_mul(
                                    pt[:, par, j * ST:(j + 1) * ST],
                                    pt[:, par, j * ST:(j + 1) * ST],
                                    mtb[:, m, :])
                    pending.append((infos, pt))
                    if len(pending) > 1:
                        emit_av(*pending.pop(0))
                    if pending_chunks and drain_every and \
                            ei % drain_every == drain_every - 1:
                        pending_chunks.pop(0)()
                for args in pending:
                    emit_av(*args)
                while pending_chunks:
                    pending_chunks.pop(0)()

                # normalization: 1/denominator broadcast via K=1 matmul
                for b_ in range(NB):
                    if not bank_slots[b_]:
                        continue
                    ltmp = p2s.tile([1, 512], F32, tag="ltmp", bufs=1)
                    nc.scalar.copy(ltmp[:, :], po[b_][HD:HD + 1, :])
                    rec = p2s.tile([1, 512], F32, tag="rec", bufs=1)
                    nc.vector.reciprocal_approx_fast(rec[:, :], ltmp[:, :])
                    rec16 = p2s.tile([1, 512], BF16, tag="rec16", bufs=1)
                    nc.scalar.copy(rec16[:, :], rec[:, :])
                    recps = pskp.tile([HD, 512], F32, tag="psk")
                    nc.tensor.matmul(recps[:, :], ones1[:, :], rec16[:, :],
                                     start=True, stop=True)
                    cslice = cat[g][r:r + HD, 4 * b_ * ST:(4 * b_ + 4) * ST]
                    nc.scalar.copy(cslice, po[b_][0:HD, :])
                    nc.vector.tensor_mul(cslice, cslice, recps[:, :])

            for g in range(NG):
                chunks = proj_chunks_for_pair(g + 1) if g + 1 < NG else []
                # split interleaved proj chunks between the two heads
                half = (len(chunks) + 1) // 2
                attention_head(g, 2 * g, chunks[:half])
                attention_head(g, 2 * g + 1, chunks[half:])

            # Wo load (staged during late attention)
            for c in range(NDC):
                for n in range(2):
                    wstg3 = p2s.tile([ST, 512], F32, tag="wstg3")
                    nc.sync.dma_start(
                        wstg3[:, :],
                        wo_d.ap()[c * ST:(c + 1) * ST,
                                  n * 512:(n + 1) * 512])
                    nc.vector.tensor_copy(wob[:, c, n * 512:(n + 1) * 512],
                                          wstg3[:, :])

            psk_cm.__exit__(None, None, None)
            po_cm.__exit__(None, None, None)
            psc_cm.__exit__(None, None, None)
            pt_cm.__exit__(None, None, None)
            p2s_cm.__exit__(None, None, None)
            wp_cm.__exit__(None, None, None)

            # ---- phase D: fc_out ---------------------------------------
            with (
                tc.tile_pool(name="p3s", bufs=3) as p3s,
                tc.tile_pool(name="psy", bufs=4, space="PSUM") as psy,
            ):
                for jt in range(NJ):
                    py = [psy.tile([ST, 512], F32, tag="py",
                                   name=f"py{jt}_{n}") for n in range(2)]
                    for c in range(NDC):
                        for n in range(2):
                            nc.tensor.matmul(
                                py[n][:, :],
                                cat[c][:, jt * ST:(jt + 1) * ST],
                                wob[:, c, n * 512:(n + 1) * 512],
                                start=(c == 0), stop=(c == NDC - 1))
                    for n in range(2):
                        ysb = p3s.tile([ST, 512], F32, tag="ysb")
                        nc.vector.tensor_add(ysb[:, :], py[n][:, :],
                                             bob[:, n * 512:(n + 1) * 512])
                        nc.sync.dma_start(
                            out_d.ap()[jt * ST:(jt + 1) * ST,
                                       n * 512:(n + 1) * 512],
                            ysb[:, :])

            xtp_cm.__exit__(None, None, None)

    nc.compile()
    return nc


_CACHE = {}
LAST_RESULT = None


def _get_program(mask):
    key = mask.tobytes()
    if key not in _CACHE:
        cls, mixed, midx, n_maskt = _classify(mask)
        _CACHE[key] = (_build(cls, mixed, midx, n_maskt), cls, mixed, midx,
                       n_maskt)
    return _CACHE[key]


def kernel(x, mask, Wq, bq, Wk, bk, Wv, bv, Wo, bo):
    x = np.ascontiguousarray(np.asarray(x, dtype=np.float32))
    mask = np.asarray(mask)
    nc, cls, mixed, midx, n_maskt = _get_program(mask)
    base = {
        "wq": np.ascontiguousarray(Wq, dtype=np.float32),
        "wk": np.ascontiguousarray(Wk, dtype=np.float32),
        "wv": np.ascontiguousarray(Wv, dtype=np.float32),
        "wo": np.ascontiguousarray(Wo, dtype=np.float32),
        "bq": np.ascontiguousarray(bq, dtype=np.float32),
        "bk": np.ascontiguousarray(bk, dtype=np.float32),
        "bv": np.ascontiguousarray(bv, dtype=np.float32),
        "bo": np.ascontiguousarray(bo, dtype=np.float32),
    }
    in_maps = []
    for c in range(N_CORES):
        b, h = c // 2, c % 2
        qrows = np.concatenate(
            [np.arange((2 * j + h) * ST, (2 * j + h + 1) * ST) for j in range(NJ)])
        mt = np.zeros((n_maskt, ST, ST), dtype=ml_dtypes.bfloat16)
        for (j, k) in mixed:
            blk = mask[(2 * j + h) * ST:(2 * j + h + 1) * ST,
                       k * ST:(k + 1) * ST]
            mt[midx[(j, k)]] = (blk != 0).T.astype(ml_dtypes.bfloat16)
        m = dict(base)
        m["x"] = x[b]
        m["xq"] = np.ascontiguousarray(x[b][qrows])
        m["maskt"] = mt
        in_maps.append(m)

    res = run_bass_kernel_spmd(
        nc, in_maps, core_ids=list(range(N_CORES)),
        trace=os.environ.get("BASS_KERNEL_TRACE", "0") == "1")
    global LAST_RESULT
    LAST_RESULT = res

    out = np.empty((B, S, D), dtype=np.float32)
    for c in range(N_CORES):
        b, h = c // 2, c % 2
        oc = res.results[c]["out"]
        for j in range(NJ):
            out[b, (2 * j + h) * ST:(2 * j + h + 1) * ST, :] = \
                oc[j * ST:(j + 1) * ST, :]
    return out
